# revision 3
# baseline (speedup 1.0000x reference)
"""APPNP GNN kernel for 8 Trainium2 NeuronCores (Bass, raw engine streams).

Node-sharded design (see module docstring history in repo):
- 50000 nodes -> 8 shards of 6250, padded to 6272 = 49 windows x 128.
- w = (1-a)*dinv[src]*dinv[dst] is rank-1 separable: shard rows are
  pre-scaled by dinv before AllGather (the "h~" table), post-scaled after
  aggregation -> no per-edge arithmetic at all.
- Per step each core dma_gathers its in-edge source rows from the
  replicated h~ table (HBM) into a window-slot layout (slot (dst p, col j)
  lands on partition p), folds slot columns per window with DVE halving
  adds, applies the APPNP combine in place, and AllGathers the new shard.
- dma_gather indices are int16, so the h~ table is addressed as two halves
  (cores 0-3 / 4-7); slot columns are grouped into a lo region and a hi
  region per window.  Pad slots point at per-shard pad rows, which the
  combine forces to zero.
- Dense layers: TensorE (transposes via host-fed identity + fp32 matmuls).
"""

import numpy as np

N = 50000
F = 128
OUT = 16
K = 10
ALPHA = 0.1
NC = 8
SH = N // NC                 # 6250
WP = 128
NW = (SH + WP - 1) // WP     # 49
SHV = NW * WP                # 6272
NV = SHV * NC                # 50176
HALF = NV // 2               # 25088
CCOL = 16                    # cols per full dma_gather call (2048 idxs)
MAXCHUNK = 96                # max cols resident (48KB/partition)
OUTP = 32


def _preprocess(edge_index):
    ei = np.asarray(edge_index)
    row = np.concatenate([ei[0], np.arange(N, dtype=np.int64)])
    col = np.concatenate([ei[1], np.arange(N, dtype=np.int64)])
    deg = np.bincount(col, minlength=N).astype(np.float64)
    dinv = 1.0 / np.sqrt(deg)

    src_lo = (row // SH) < (NC // 2)
    Lcnt = np.bincount(col[src_lo], minlength=N)
    Hcnt = np.bincount(col[~src_lo], minlength=N)

    pos = np.empty(N, dtype=np.int64)
    perm = []
    for c in range(NC):
        nodes = np.arange(c * SH, (c + 1) * SH)
        nodes = nodes[np.lexsort((Hcnt[nodes], Lcnt[nodes]))]
        perm.append(nodes)
        pos[nodes] = c * SHV + np.arange(SH)

    e_dst_pos = pos[col]
    order = np.argsort(e_dst_pos, kind="stable")
    sp_all = pos[row][order]
    dp_all = e_dst_pos[order]
    lo_all = sp_all < HALF

    maxL = np.zeros(NW, dtype=np.int64)
    maxH = np.zeros(NW, dtype=np.int64)
    per_core = []
    for c in range(NC):
        s = np.searchsorted(dp_all, c * SHV)
        e = np.searchsorted(dp_all, c * SHV + SH)
        dp = dp_all[s:e] - c * SHV
        sp = sp_all[s:e]
        lo = lo_all[s:e]
        lc = np.bincount(dp[lo], minlength=SHV).reshape(NW, WP)
        hc = np.bincount(dp[~lo], minlength=SHV).reshape(NW, WP)
        maxL = np.maximum(maxL, lc.max(axis=1))
        maxH = np.maximum(maxH, hc.max(axis=1))
        per_core.append((sp, dp, lo))

    maxL = np.maximum(((maxL + 3) // 4) * 4, 4)
    maxH = np.maximum(((maxH + 3) // 4) * 4, 4)

    # region layouts: lo region cols then hi region cols
    offL = np.zeros(NW + 1, dtype=np.int64)
    offL[1:] = np.cumsum(maxL)
    offH = np.zeros(NW + 1, dtype=np.int64)
    offH[1:] = np.cumsum(maxH)
    nlo = int(offL[-1])
    ncols = nlo + int(offH[-1])

    # chunks: (region_is_lo, w0, nwc, C, col0)
    def mk_chunks(maxC, off, base, is_lo):
        out = []
        w = 0
        while w < NW:
            C = int(maxC[w])
            nw = 1
            while (w + nw < NW and maxC[w + nw] == C
                   and (nw + 1) * C <= MAXCHUNK):
                nw += 1
            out.append((is_lo, w, nw, C, base + int(off[w])))
            w += nw
        return out

    chunks = mk_chunks(maxL, offL, 0, True) + mk_chunks(maxH, offH, nlo, False)

    ZLO = SH            # pad row of shard 0 (lo table local)
    ZHI = SH            # pad row of shard 4 (hi table local)

    idx_maps = []
    for c in range(NC):
        sp, dp, lo = per_core[c]
        slots = np.empty((ncols, WP), dtype=np.int64)
        slots[:nlo] = ZLO
        slots[nlo:] = ZHI
        win = dp // WP
        prow = dp % WP
        for half in (True, False):
            m = lo == half
            dpm = dp[m]
            first = np.r_[True, dpm[1:] != dpm[:-1]]
            start = np.maximum.accumulate(
                np.where(first, np.arange(dpm.size), 0))
            rank = np.arange(dpm.size) - start
            if half:
                cols = offL[win[m]] + rank
                src_local = sp[m]
            else:
                cols = nlo + offH[win[m]] + rank
                src_local = sp[m] - HALF
            slots[cols, prow[m]] = src_local
        flat = slots.reshape(-1)                      # pos i = col*128 + p
        idx_maps.append(np.ascontiguousarray(
            flat.reshape(-1, 16).T.astype(np.int16)))  # [16, ncols*8]

    return dinv, perm, ncols, chunks, idx_maps


def _build(chunks, ncols):
    from concourse import bass, bacc, mybir

    DT = mybir.dt.float32
    I16 = mybir.dt.int16
    AL = mybir.AluOpType
    AX = mybir.AxisListType
    ACT = mybir.ActivationFunctionType

    nc = bacc.Bacc("TRN2", target_bir_lowering=False, debug=False,
                   num_devices=NC)

    x_d = nc.dram_tensor("x", [SHV, F], DT, kind="ExternalInput")
    W_d = [nc.dram_tensor(f"W{i}", [F, F], DT, kind="ExternalInput")
           for i in (1, 2, 3)]
    b_d = [nc.dram_tensor(f"b{i}", [F, 1], DT, kind="ExternalInput")
           for i in (1, 2, 3)]
    Wc_d = nc.dram_tensor("Wc", [F, OUTP], DT, kind="ExternalInput")
    bc_d = nc.dram_tensor("bc", [OUTP, 1], DT, kind="ExternalInput")
    idx_d = nc.dram_tensor("idx", [16, ncols * 8], I16, kind="ExternalInput")
    ident_d = nc.dram_tensor("ident", [128, 128], DT, kind="ExternalInput")
    dinvm_d = nc.dram_tensor("dinvm", [128, NW], DT, kind="ExternalInput")
    sm_d = nc.dram_tensor("sm", [128, NW], DT, kind="ExternalInput")
    spm_d = nc.dram_tensor("spm", [128, NW], DT, kind="ExternalInput")
    am_d = nc.dram_tensor("am", [128, NW], DT, kind="ExternalInput")
    out_d = nc.dram_tensor("out", [SHV, OUTP], DT, kind="ExternalOutput")

    hsh_d = nc.dram_tensor("hsh", [SHV, F], DT)
    hfull_d = nc.dram_tensor("hfull", [NV, F], DT, addr_space="Shared")

    ix = nc.alloc_sbuf_tensor("ix", [128, ncols * 8], I16)
    slots = nc.alloc_sbuf_tensor("slots", [128, MAXCHUNK, F], DT)
    agg = nc.alloc_sbuf_tensor("agg", [128, NW, F], DT)
    az = nc.alloc_sbuf_tensor("az", [128, NW, F], DT)
    S = nc.alloc_sbuf_tensor("S", [128, NW, F], DT)
    hT = nc.alloc_sbuf_tensor("hT", [128, NW, 128], DT)
    zT = nc.alloc_sbuf_tensor("zT", [128, NW, 128], DT)
    Wsb = nc.alloc_sbuf_tensor("Wsb", [F, F], DT)
    bsb = nc.alloc_sbuf_tensor("bsb", [F, 1], DT)
    Wcsb = nc.alloc_sbuf_tensor("Wcsb", [F, OUTP], DT)
    bcsb = nc.alloc_sbuf_tensor("bcsb", [OUTP, 1], DT)
    ident = nc.alloc_sbuf_tensor("ident_s", [128, 128], DT)
    lg = nc.alloc_sbuf_tensor("lg", [128, NW, OUTP], DT)
    red = nc.alloc_sbuf_tensor("red", [128, NW, 1], DT)
    dinvm = nc.alloc_sbuf_tensor("dinvm_s", [128, NW], DT)
    sm = nc.alloc_sbuf_tensor("sm_s", [128, NW], DT)
    spm = nc.alloc_sbuf_tensor("spm_s", [128, NW], DT)
    am = nc.alloc_sbuf_tensor("am_s", [128, NW], DT)

    psA = nc.alloc_psum_tensor("psA", [128, 512], DT)
    psB = nc.alloc_psum_tensor("psB", [128, 512], DT)
    psT = nc.alloc_psum_tensor("psT", [128, 128], DT)
    psU = nc.alloc_psum_tensor("psU", [128, 128], DT)

    rg = [list(range(NC))]
    prog = {e: [] for e in ("sync", "gpsimd", "vector", "tensor", "scalar")}
    cnt = dict(d=0, g=0, v=0, t=0, s=0, c=0)
    sems = {}
    psv = {}          # psum tensor name -> v count of last consumer copy

    def em(engine, fn):
        prog[engine].append(fn)

    def wait(engine, s, val):
        if val > 0:
            em(engine, lambda e, s=s, v=val: e.wait_ge(sems[s], v))

    def dma(out_ap, in_ap):
        cnt["d"] += 1
        em("sync", lambda e, o=out_ap, i=in_ap:
           e.dma_start(out=o, in_=i).then_inc(sems["d"], 16))
        return cnt["d"]

    def vop(fn):
        cnt["v"] += 1
        em("vector", lambda e, f=fn: f(e).then_inc(sems["v"], 1))
        return cnt["v"]

    def top(fn):
        cnt["t"] += 1
        em("tensor", lambda e, f=fn: f(e).then_inc(sems["t"], 1))
        return cnt["t"]

    def sop(fn):
        cnt["s"] += 1
        em("scalar", lambda e, f=fn: f(e).then_inc(sems["s"], 1))
        return cnt["s"]

    def bcast(m, n):
        return m[:, :, None].broadcast_to([128, NW, n])

    # ---- constant loads ----
    for r in range(8):
        dma(ix[16 * r:16 * (r + 1), :], idx_d[:])
    for (dst, src) in [(ident, ident_d), (dinvm, dinvm_d), (sm, sm_d),
                       (spm, spm_d), (am, am_d), (Wcsb, Wc_d), (bcsb, bc_d)]:
        dma(dst[:], src[:])
    dma(S[:], x_d[:].rearrange("(w p) f -> p w f", p=128))

    def transpose_128(dst_ap, src_ap, ps, pn=128, fn=128):
        """dst[fn,pn] = src[pn,fn].T via TensorE."""
        wait("tensor", "v", psv.get(ps.name, 0))
        tn = top(lambda e, o=ps, i=src_ap, pn=pn, fn=fn: e.matmul(
            out=o.ap()[0:fn, 0:pn], lhsT=i, rhs=ident[0:pn, 0:pn],
            start=True, stop=True, is_transpose=True))
        wait("vector", "t", tn)
        vn = vop(lambda e, o=dst_ap, i=ps, pn=pn, fn=fn:
                 e.tensor_copy(out=o, in_=i.ap()[0:fn, 0:pn]))
        psv[ps.name] = vn
        return vn

    def fold_chunk(is_lo, w0, nwc, C, agg_accum):
        """fold slots[:, :nwc*C, :] into agg[:, w0:w0+nwc, :].
        agg_accum: False -> overwrite agg, True -> add into agg."""
        v = slots[:, 0:nwc * C, :].rearrange("p (w c) f -> p w c f", w=nwc)
        width = C
        while width > 2:
            h = width // 2
            vop(lambda e, o=v[:, :, 0:h, :], i1=v[:, :, width - h:width, :]:
                e.tensor_tensor(o, o, i1, AL.add))
            width -= h
        a = agg[:, w0:w0 + nwc, :]
        v0 = v[:, :, 0:1, :].rearrange("p w c f -> p (w c) f")
        v1 = v[:, :, 1:2, :].rearrange("p w c f -> p (w c) f")
        if not agg_accum:
            vop(lambda e, a=a, v0=v0, v1=v1: e.tensor_tensor(a, v0, v1, AL.add))
        else:
            vop(lambda e, v0=v0, v1=v1: e.tensor_tensor(v0, v0, v1, AL.add))
            vop(lambda e, a=a, v0=v0: e.tensor_tensor(a, a, v0, AL.add))

    def dense(blk):
        src = S if blk == 0 else S      # relu result is written into S
        wait("sync", "t", cnt["t"])     # don't clobber Wsb mid-use
        dma(Wsb[:], W_d[blk][:])
        dma(bsb[:], b_d[blk][:])
        d_need = cnt["d"]
        wait("tensor", "d", 16 * d_need)
        wait("vector", "d", 16 * d_need)
        wait("tensor", "v", cnt["v"])   # S ready (combine/relu done)
        pss = [psT, psU]
        for w in range(NW):
            transpose_128(hT[:, w, :], src[:, w, :], pss[w % 2])
        hTflat = hT[:].rearrange("p w f -> p (w f)")
        zTflat = zT[:].rearrange("p w f -> p (w f)")
        ps2 = [psA, psB]
        v_hT = cnt["v"]
        for j in range(13):
            n0 = j * 512
            n1 = min((j + 1) * 512, SHV)
            ps = ps2[j % 2]
            wait("tensor", "v", max(psv.get(ps.name, 0), v_hT))
            tn = top(lambda e, o=ps, i=hTflat[:, n0:n1], n=n1 - n0: e.matmul(
                out=o.ap()[:, 0:n], lhsT=Wsb[:], rhs=i,
                start=True, stop=True))
            wait("vector", "t", tn)
            vn = vop(lambda e, o=zTflat[:, n0:n1], i=ps, n=n1 - n0:
                     e.tensor_tensor(o, i.ap()[:, 0:n],
                                     bsb[:, 0:1].broadcast_to([128, n]),
                                     AL.add))
            psv[ps.name] = vn
        for w in range(NW):
            transpose_128(S[:, w, :], zT[:, w, :], pss[w % 2])
            vop(lambda e, w=w: e.tensor_tensor(
                az[:, w, :], S[:, w, :],
                am[:, w:w + 1].broadcast_to([128, F]), AL.mult))
            vop(lambda e, w=w: e.tensor_tensor(
                agg[:, w, :], S[:, w, :],
                dinvm[:, w:w + 1].broadcast_to([128, F]), AL.mult))

    def ship_and_allgather():
        wait("sync", "v", cnt["v"])
        wait("sync", "c", cnt["c"])     # prior AG finished reading hsh
        dma(hsh_d[:].rearrange("(w p) f -> p w f", p=128), agg[:])
        wait("gpsimd", "d", 16 * cnt["d"])
        cnt["c"] += 1
        em("gpsimd", lambda e: e.collective_compute(
            "AllGather", AL.bypass, replica_groups=rg,
            ins=[hsh_d[:]], outs=[hfull_d[:]]).then_inc(sems["c"], 1))

    # ---------------- schedule ----------------
    for blk in range(3):
        dense(blk)
        ship_and_allgather()
        for k in range(K):
            last = (k == K - 1)
            wait("gpsimd", "c", cnt["c"])
            first_lo_chunk = True
            first_hi = {}
            for (ci, (is_lo, w0, nwc, C, col0)) in enumerate(chunks):
                wait("gpsimd", "v", cnt["v"])    # slots free (prev folds)
                g_first = cnt["g"]
                ncc_total = nwc * C
                off = 0
                while off < ncc_total:
                    take = min(CCOL, ncc_total - off)
                    c0 = col0 + off
                    nidx = take * WP
                    table = hfull_d[0:HALF] if is_lo else hfull_d[HALF:NV]
                    cnt["g"] += 1
                    em("gpsimd", lambda e, o=slots[:, off:off + take, :],
                       t=table, ii=ix[:, c0 * 8:(c0 + take) * 8], n=nidx:
                       e.dma_gather(out_ap=o, in_ap=t, idxs_ap=ii,
                                    num_idxs=n, num_idxs_reg=n,
                                    elem_size=F, single_packet=False)
                       .then_inc(sems["g"], 16))
                    wait("gpsimd", "g", 16 * cnt["g"])
                    off += take
                wait("vector", "g", 16 * cnt["g"])
                fold_chunk(is_lo, w0, nwc, C, agg_accum=not is_lo)
            # combine
            if not last:
                # h~ = agg*spm + az*dinvm
                vop(lambda e: e.tensor_tensor(agg[:], agg[:],
                                              bcast(spm, F), AL.mult))
                vop(lambda e: e.tensor_tensor(zT[:, 0:NW, :], az[:],
                                              bcast(dinvm, F), AL.mult))
                vop(lambda e: e.tensor_tensor(agg[:], agg[:],
                                              zT[:, 0:NW, :], AL.add))
                ship_and_allgather()
            else:
                # h = agg*sm + az ; S = relu(h)
                vop(lambda e: e.tensor_tensor(agg[:], agg[:],
                                              bcast(sm, F), AL.mult))
                vop(lambda e: e.tensor_tensor(agg[:], agg[:], az[:], AL.add))
                vop(lambda e: e.tensor_scalar_max(S[:], agg[:], 0.0))

    # ---------------- head ----------------
    pss = [psT, psU]
    wait("tensor", "v", cnt["v"])
    for w in range(NW):
        transpose_128(hT[:, w, :], S[:, w, :], pss[w % 2])
    hTflat = hT[:].rearrange("p w f -> p (w f)")
    lT = zT          # reuse as [32, nodes] logits buffer
    lTflat = lT[:].rearrange("p w f -> p (w f)")
    ps2 = [psA, psB]
    v_hT = cnt["v"]
    for j in range(13):
        n0 = j * 512
        n1 = min((j + 1) * 512, SHV)
        ps = ps2[j % 2]
        wait("tensor", "v", max(psv.get(ps.name, 0), v_hT))
        tn = top(lambda e, o=ps, i=hTflat[:, n0:n1], n=n1 - n0: e.matmul(
            out=o.ap()[0:OUTP, 0:n], lhsT=Wcsb[:], rhs=i,
            start=True, stop=True))
        wait("vector", "t", tn)
        vn = vop(lambda e, o=lTflat[0:OUTP, n0:n1], i=ps, n=n1 - n0:
                 e.tensor_tensor(o, i.ap()[0:OUTP, 0:n],
                                 bcsb[:, 0:1].broadcast_to([OUTP, n]),
                                 AL.add))
        psv[ps.name] = vn
    for w in range(NW):
        transpose_128(lg[:, w, :], lT[0:OUTP, w, :], pss[w % 2],
                      pn=OUTP, fn=128)
    vop(lambda e: e.tensor_reduce(out=red[:], in_=lg[:, :, 0:OUT],
                                  axis=AX.X, op=AL.max))
    vop(lambda e: e.tensor_tensor(lg[:, :, 0:OUT], lg[:, :, 0:OUT],
                                  red[:].broadcast_to([128, NW, OUT]),
                                  AL.subtract))
    wait("scalar", "v", cnt["v"])
    sn = sop(lambda e: e.activation(out=az[:, :, 0:OUT], in_=lg[:, :, 0:OUT],
                                    func=ACT.Exp))
    wait("vector", "s", sn)
    vop(lambda e: e.tensor_reduce(out=red[:], in_=az[:, :, 0:OUT],
                                  axis=AX.X, op=AL.add))
    wait("scalar", "v", cnt["v"])
    sn = sop(lambda e: e.activation(out=red[:], in_=red[:], func=ACT.Ln))
    wait("vector", "s", sn)
    vop(lambda e: e.tensor_tensor(lg[:, :, 0:OUT], lg[:, :, 0:OUT],
                                  red[:].broadcast_to([128, NW, OUT]),
                                  AL.subtract))
    wait("sync", "v", cnt["v"])
    dma(out_d[:].rearrange("(w p) f -> p w f", p=128), lg[:])

    d_total, g_total, c_total = cnt["d"], cnt["g"], cnt["c"]

    with (
        nc.Block() as block,
        nc.semaphore("d_sem") as d_sem,
        nc.semaphore("g_sem") as g_sem,
        nc.semaphore("v_sem") as v_sem,
        nc.semaphore("t_sem") as t_sem,
        nc.semaphore("s_sem") as s_sem,
        nc.semaphore("c_sem") as c_sem,
    ):
        sems.update(d=d_sem, g=g_sem, v=v_sem, t=t_sem, s=s_sem, c=c_sem)

        @block.sync
        def _(eng):
            for fn in prog["sync"]:
                fn(eng)
            eng.wait_ge(d_sem, 16 * d_total)

        @block.gpsimd
        def _(eng):
            for fn in prog["gpsimd"]:
                fn(eng)
            eng.wait_ge(g_sem, 16 * g_total)
            eng.wait_ge(c_sem, c_total)

        @block.vector
        def _(eng):
            for fn in prog["vector"]:
                fn(eng)

        @block.tensor
        def _(eng):
            for fn in prog["tensor"]:
                fn(eng)

        @block.scalar
        def _(eng):
            for fn in prog["scalar"]:
                fn(eng)

    nc.compile()
    return nc


LAST_RES = None


def kernel(x, edge_index, W1, b1, W2, b2, W3, b3, Wc, bc):
    from concourse.bass_utils import run_bass_kernel_spmd

    dinv, perm, ncols, chunks, idx_maps = _preprocess(edge_index)
    nc = _build(chunks, ncols)

    Wc_pad = np.zeros((F, OUTP), np.float32)
    Wc_pad[:, :OUT] = np.asarray(Wc)
    bc_pad = np.zeros((OUTP, 1), np.float32)
    bc_pad[:OUT, 0] = np.asarray(bc)
    ident = np.eye(128, dtype=np.float32)

    in_maps = []
    for c in range(NC):
        xs = np.zeros((SHV, F), np.float32)
        xs[:SH] = np.asarray(x)[perm[c]]
        dv = np.zeros(SHV, np.float64)
        dv[:SH] = dinv[perm[c]]
        mask = np.zeros(SHV, np.float64)
        mask[:SH] = 1.0

        def pm(vec):
            return np.ascontiguousarray(
                vec.reshape(NW, WP).T.astype(np.float32))

        im = dict(x=xs,
                  W1=np.asarray(W1, np.float32),
                  b1=np.asarray(b1, np.float32).reshape(F, 1),
                  W2=np.asarray(W2, np.float32),
                  b2=np.asarray(b2, np.float32).reshape(F, 1),
                  W3=np.asarray(W3, np.float32),
                  b3=np.asarray(b3, np.float32).reshape(F, 1),
                  Wc=Wc_pad, bc=bc_pad, idx=idx_maps[c], ident=ident,
                  dinvm=pm(dv),
                  sm=pm((1.0 - ALPHA) * dv),
                  spm=pm((1.0 - ALPHA) * dv * dv),
                  am=pm(ALPHA * mask))
        in_maps.append(im)

    res = run_bass_kernel_spmd(nc, in_maps, core_ids=list(range(NC)))
    global LAST_RES
    LAST_RES = res
    out = np.empty((N, OUT), np.float32)
    for c in range(NC):
        out[perm[c]] = res.results[c]["out"][:SH, :OUT]
    return out

